# revision 17
# baseline (speedup 1.0000x reference)
"""Trainium2 Bass kernel for a dense transformer block (B=2, T=4096, C=512, H=8).

Strategy: token-parallel across 8 NeuronCores with causal load balancing.
Core i handles batch b=i//4 and the eight 128-token query blocks
{8g+c, 8g+7-c : g=0..3} with c=i%4, so every core performs identical causal
attention work -> fully static SPMD program, no collectives. k/v are computed
per batch-group from the full per-batch input (replicated compute instead of
communication). LayerNorm gains/shifts are folded into the weight matrices on
the host; matmuls run in bf16 with fp32 accumulation; softmax denominators
come from a ones-column appended to V.
"""

import sys

sys.path.insert(0, "/opt/trn_rl_repo")

from contextlib import ExitStack

import ml_dtypes
import numpy as np

import concourse.bass as bass
import concourse.tile as tile
from concourse import bacc, mybir
from concourse.bass_utils import run_bass_kernel_spmd
from concourse.masks import make_identity

F32 = mybir.dt.float32
F32R = mybir.dt.float32r
BF16 = mybir.dt.bfloat16
BF16NP = ml_dtypes.bfloat16

B, T, C, H = 2, 4096, 512, 8
D = C // H            # 64
FF = 4 * C            # 2048
EPS = 1e-5
NB = T // 128         # 32 token blocks per batch
QB = 8                # query blocks per core
QT = QB * 128         # 1024 query tokens per core
NCORES = 8

Exp = mybir.ActivationFunctionType.Exp
Sqrt = mybir.ActivationFunctionType.Sqrt
Identity = mybir.ActivationFunctionType.Identity
Relu = mybir.ActivationFunctionType.Relu
SUB = mybir.AluOpType.subtract
MULT = mybir.AluOpType.mult


def _bcast_ap(ap1d, p=128):
    """[N] dram AP -> stride-0 partition broadcast [p, N]."""
    return bass.AP(tensor=ap1d.tensor, offset=ap1d.offset,
                   ap=[[0, p]] + [list(d) for d in ap1d.ap])


DEBUG = False


def build_nc():
    nc = bacc.Bacc("TRN2", target_bir_lowering=False, debug=False,
                   num_devices=NCORES)

    x_full = nc.dram_tensor("x_full", [T, C], F32, kind="ExternalInput").ap()
    xq = nc.dram_tensor("xq", [QT, C], F32, kind="ExternalInput").ap()
    wq_d = nc.dram_tensor("wq", [C, C], BF16, kind="ExternalInput").ap()
    wk_d = nc.dram_tensor("wk", [C, C], BF16, kind="ExternalInput").ap()
    wv_d = nc.dram_tensor("wv", [C, C], BF16, kind="ExternalInput").ap()
    wo_d = nc.dram_tensor("wo", [C, C], BF16, kind="ExternalInput").ap()
    w1_d = nc.dram_tensor("w1", [C, FF], BF16, kind="ExternalInput").ap()
    w2_d = nc.dram_tensor("w2", [FF, C], BF16, kind="ExternalInput").ap()
    bq_d = nc.dram_tensor("bq", [C], F32, kind="ExternalInput").ap()
    bk_d = nc.dram_tensor("bk", [C], F32, kind="ExternalInput").ap()
    bv_d = nc.dram_tensor("bv", [C], F32, kind="ExternalInput").ap()
    bo_d = nc.dram_tensor("bo", [C], F32, kind="ExternalInput").ap()
    b1_d = nc.dram_tensor("b1", [FF], F32, kind="ExternalInput").ap()
    b2_d = nc.dram_tensor("b2", [C], F32, kind="ExternalInput").ap()
    mask_d = nc.dram_tensor("masks", [8, 128, 256], BF16, kind="ExternalInput").ap()
    out_d = nc.dram_tensor("out", [QT, C], F32, kind="ExternalOutput").ap()
    if DEBUG:
        dbg_sc = nc.dram_tensor("dbg_sc", [8, 128, QT], F32, kind="ExternalOutput").ap()
        dbg_es = nc.dram_tensor("dbg_es", [8, 128, QT], F32, kind="ExternalOutput").ap()
        dbg_den = nc.dram_tensor("dbg_den", [1, QT], F32, kind="ExternalOutput").ap()
        dbg_num = nc.dram_tensor("dbg_num", [64, QT], F32, kind="ExternalOutput").ap()
        dbg_acc = nc.dram_tensor("dbg_acc", [8, 65, 256], F32, kind="ExternalOutput").ap()

    with tile.TileContext(nc) as tc, ExitStack() as top:
        consts = top.enter_context(tc.tile_pool(name="consts", bufs=1))

        ident = consts.tile([128, 128], BF16)
        make_identity(nc, ident)
        eps_t = consts.tile([128, 1], F32)
        nc.vector.memset(eps_t, EPS)
        onesf = consts.tile([1, 64], F32)
        nc.vector.memset(onesf, 1.0)
        onesr = consts.tile([1, 64], F32R)
        nc.vector.tensor_copy(out=onesr, in_=onesf)

        bq_sb = consts.tile([128, 4], F32)
        nc.sync.dma_start(out=bq_sb, in_=bq_d.rearrange("(g p) -> p g", p=128))
        bk_sb = consts.tile([128, 4], F32)
        nc.sync.dma_start(out=bk_sb, in_=bk_d.rearrange("(g p) -> p g", p=128))
        b1_sb = consts.tile([128, 16], F32)
        nc.sync.dma_start(out=b1_sb, in_=b1_d.rearrange("(g p) -> p g", p=128))
        bvB = consts.tile([128, C], F32)
        nc.sync.dma_start(out=bvB, in_=_bcast_ap(bv_d))
        boB = consts.tile([128, C], F32)
        nc.sync.dma_start(out=boB, in_=_bcast_ap(bo_d))
        b2B = consts.tile([128, C], F32)
        nc.sync.dma_start(out=b2B, in_=_bcast_ap(b2_d))
        masks_sb = consts.tile([128, 8, 256], BF16)
        nc.sync.dma_start(out=masks_sb, in_=mask_d.rearrange("j p c -> p j c"))

        # resident activations whose lifetime spans multiple phases
        res_xq = top.enter_context(tc.tile_pool(name="res_xq", bufs=1))
        xq_sb = res_xq.tile([128, QB, C], F32)
        nc.sync.dma_start(out=xq_sb, in_=xq.rearrange("(q p) c -> p q c", p=128))

        def layernorm_block(work, x_sb_slice, h_out_slice):
            """token-major LN of [128, C] -> bf16 into h_out_slice."""
            stats = work.tile([128, 6], F32, tag="ln_stats")
            nc.vector.bn_stats(out=stats, in_=x_sb_slice)
            mv = work.tile([128, 2], F32, tag="ln_mv")
            nc.vector.bn_aggr(out=mv, in_=stats)
            rs = work.tile([128, 1], F32, tag="ln_rs")
            nc.scalar.activation(out=rs, in_=mv[:, 1:2], func=Sqrt, bias=eps_t,
                                 scale=1.0)
            nc.vector.reciprocal(out=rs, in_=rs)
            nc.vector.tensor_scalar(out=h_out_slice, in0=x_sb_slice,
                                    scalar1=mv[:, 0:1], scalar2=rs,
                                    op0=SUB, op1=MULT)

        # ---------------- Phase A+B: LN1, transpose, projections ----------
        es_hT = ExitStack()
        es_att = ExitStack()
        es_attnT = ExitStack()
        es_d = ExitStack()
        if True:
            res_att = es_att.enter_context(
                tc.tile_pool(name="res_att", bufs=1, side="right"))
            res_hT = es_hT.enter_context(tc.tile_pool(name="res_hT", bufs=1))
            hT = res_hT.tile([128, 4, T], BF16)
            hqT = res_hT.tile([128, 4, QT], BF16)
            kT = res_att.tile([128, 4, T], BF16)
            v_sb = res_att.tile([128, NB, H, D + 1], BF16)
            qT = res_att.tile([128, 4, QT], BF16)

            with tc.tile_pool(name="a_work", bufs=3) as work, \
                 tc.tile_pool(name="a_psum", bufs=4, space="PSUM") as apsum:
                for tb in range(NB):
                    x_sb = work.tile([128, C], F32, tag="x_in")
                    nc.sync.dma_start(out=x_sb, in_=x_full[tb * 128:(tb + 1) * 128, :])
                    h_bf = work.tile([128, C], BF16, tag="h_bf")
                    layernorm_block(work, x_sb, h_bf)
                    for cc in range(4):
                        tp = apsum.tile([128, 128], BF16, tag="tp")
                        nc.tensor.transpose(tp, h_bf[:, cc * 128:(cc + 1) * 128], ident)
                        nc.vector.tensor_copy(
                            out=hT[:, cc, tb * 128:(tb + 1) * 128], in_=tp)
                for qb in range(QB):
                    h_bf = work.tile([128, C], BF16, tag="h_bf")
                    layernorm_block(work, xq_sb[:, qb, :], h_bf)
                    for cc in range(4):
                        tp = apsum.tile([128, 128], BF16, tag="tp")
                        nc.tensor.transpose(tp, h_bf[:, cc * 128:(cc + 1) * 128], ident)
                        nc.vector.tensor_copy(
                            out=hqT[:, cc, qb * 128:(qb + 1) * 128], in_=tp)

            with tc.tile_pool(name="b_w", bufs=1) as bw, \
                 tc.tile_pool(name="b_psum", bufs=4, space="PSUM") as bpsum:
                wq_sb = bw.tile([128, 4, C], BF16)
                nc.sync.dma_start(out=wq_sb, in_=wq_d.rearrange("(g p) o -> p g o", p=128))
                wk_sb = bw.tile([128, 4, C], BF16)
                nc.sync.dma_start(out=wk_sb, in_=wk_d.rearrange("(g p) o -> p g o", p=128))
                wv_sb = bw.tile([128, 4, C], BF16)
                nc.sync.dma_start(out=wv_sb, in_=wv_d.rearrange("(g p) o -> p g o", p=128))

                # kT[f, t] over full batch; qT[f, t] over this core's tokens
                for hp in range(4):
                    for ts in range(T // 512):
                        ps = bpsum.tile([128, 512], F32, tag="proj")
                        for cc in range(4):
                            nc.tensor.matmul(
                                ps, wk_sb[:, cc, hp * 128:(hp + 1) * 128],
                                hT[:, cc, ts * 512:(ts + 1) * 512],
                                start=(cc == 0), stop=(cc == 3))
                        nc.scalar.activation(
                            out=kT[:, hp, ts * 512:(ts + 1) * 512], in_=ps,
                            func=Identity, bias=bk_sb[:, hp:hp + 1], scale=1.0)
                    for ts in range(QT // 512):
                        ps = bpsum.tile([128, 512], F32, tag="proj")
                        for cc in range(4):
                            nc.tensor.matmul(
                                ps, wq_sb[:, cc, hp * 128:(hp + 1) * 128],
                                hqT[:, cc, ts * 512:(ts + 1) * 512],
                                start=(cc == 0), stop=(cc == 3))
                        nc.scalar.activation(
                            out=qT[:, hp, ts * 512:(ts + 1) * 512], in_=ps,
                            func=Identity, bias=bq_sb[:, hp:hp + 1], scale=1.0)

                # v token-major [tok, h, d] + ones column for the denominators
                for tb in range(NB):
                    ps = bpsum.tile([128, 512], F32, tag="proj")
                    for cc in range(4):
                        nc.tensor.matmul(
                            ps, hT[:, cc, tb * 128:(tb + 1) * 128],
                            wv_sb[:, cc, :], start=(cc == 0), stop=(cc == 3))
                    nc.vector.tensor_add(
                        out=v_sb[:, tb, :, 0:D],
                        in0=ps.rearrange("p (h d) -> p h d", h=H),
                        in1=bvB.rearrange("p (h d) -> p h d", h=H))
                    nc.vector.memset(v_sb[:, tb, :, D:D + 1], 1.0)

            es_hT.close()  # hT/hqT no longer needed past the projections

            # ---------------- Phase C: attention ---------------------------
            if True:
                res_attnT = es_attnT.enter_context(
                    tc.tile_pool(name="res_attnT", bufs=1))
                attnT = res_attnT.tile([128, 4, QT], BF16)
                with tc.tile_pool(name="c_work", bufs=3) as cwork, \
                     tc.tile_pool(name="c_small", bufs=3) as csmall, \
                     tc.tile_pool(name="c_psum_sc", bufs=2, space="PSUM") as cps_sc, \
                     tc.tile_pool(name="c_psum_av", bufs=2, space="PSUM") as cps_av:
                    for h in range(H):
                        hp, r0 = h // 2, 64 * (h % 2)
                        attv = cps_av.tile([65, QT], F32, tag="attv")
                        for sb in range(NB):
                            gd = sb // 8
                            cmin = 256 * gd
                            lhs_k = kT[r0:r0 + 64, hp, sb * 128:(sb + 1) * 128]
                            sc = cps_sc.tile([128, QT], F32, tag="sc")
                            if cmin < 512:
                                nc.tensor.matmul(sc[:, cmin:512], lhs_k,
                                                 qT[r0:r0 + 64, hp, cmin:512],
                                                 start=True, stop=True)
                            lo = max(512, cmin)
                            nc.tensor.matmul(sc[:, lo:QT], lhs_k,
                                             qT[r0:r0 + 64, hp, lo:QT],
                                             start=True, stop=True)
                            es = cwork.tile([128, QT], BF16, tag="es")
                            nc.scalar.activation(out=es[:, cmin:QT],
                                                 in_=sc[:, cmin:QT], func=Exp)
                            nc.vector.tensor_mul(
                                out=es[:, cmin:cmin + 256],
                                in0=es[:, cmin:cmin + 256],
                                in1=masks_sb[:, sb % 8, :])
                            if DEBUG and h == 0 and sb < 8:
                                t1 = cwork.tile([128, QT], F32, tag="dbg1")
                                nc.vector.tensor_copy(out=t1, in_=sc)
                                nc.sync.dma_start(out=dbg_sc[sb], in_=t1)
                                t2 = cwork.tile([128, QT], F32, tag="dbg2")
                                nc.vector.tensor_copy(out=t2, in_=es)
                                nc.sync.dma_start(out=dbg_es[sb], in_=t2)
                            lhs_v = v_sb[:, sb, h, :]
                            if sb == 0:
                                # exactly one start=True MM per psum bank:
                                # start clears has_written for the WHOLE bank,
                                # so bank A must be initialized by a single
                                # [0:512) matmul, not two region writes.
                                nc.tensor.matmul(attv[:, 0:512], lhs_v,
                                                 es[:, 0:512],
                                                 start=True, stop=False)
                                nc.tensor.matmul(attv[:, 512:QT], lhs_v,
                                                 es[:, 512:QT],
                                                 start=True, stop=False)
                            else:
                                # diagonal group's 256 columns (this group
                                # stops at sb == 8*gd+7), then full-valid tails
                                nc.tensor.matmul(
                                    attv[:, cmin:cmin + 256], lhs_v,
                                    es[:, cmin:cmin + 256],
                                    start=False, stop=(sb == 8 * gd + 7))
                                if cmin + 256 < 512:
                                    nc.tensor.matmul(attv[:, cmin + 256:512],
                                                     lhs_v, es[:, cmin + 256:512],
                                                     start=False, stop=False)
                                hi = max(512, cmin + 256)
                                if hi < QT:
                                    nc.tensor.matmul(attv[:, hi:QT], lhs_v,
                                                     es[:, hi:QT],
                                                     start=False, stop=(sb == NB - 1))
                            if DEBUG and h == 0 and sb < 8:
                                t5 = cwork.tile([65, 256], F32, tag="dbg5")
                                nc.vector.tensor_copy(out=t5, in_=attv[:, 0:256])
                                nc.sync.dma_start(out=dbg_acc[sb], in_=t5)
                        if DEBUG and h == 0:
                            t3 = cwork.tile([1, QT], F32, tag="dbg3")
                            nc.vector.tensor_copy(out=t3, in_=attv[64:65, :])
                            nc.sync.dma_start(out=dbg_den[:, :], in_=t3)
                            t4 = cwork.tile([64, QT], F32, tag="dbg4")
                            nc.vector.tensor_copy(out=t4, in_=attv[0:64, :])
                            nc.sync.dma_start(out=dbg_num[:, :], in_=t4)
                        rcp = csmall.tile([1, QT], F32, tag="rcp")
                        nc.vector.reciprocal(out=rcp, in_=attv[64:65, :])
                        rcpr = csmall.tile([1, QT], F32R, tag="rcpr")
                        nc.vector.tensor_copy(out=rcpr, in_=rcp)
                        rb = cps_sc.tile([64, QT], F32, tag="sc")
                        nc.tensor.matmul(rb[:, 0:512], onesr, rcpr[:, 0:512],
                                         start=True, stop=True)
                        nc.tensor.matmul(rb[:, 512:QT], onesr, rcpr[:, 512:QT],
                                         start=True, stop=True)
                        rbs = cwork.tile([64, QT], F32, tag="rbs")
                        nc.vector.tensor_copy(out=rbs, in_=rb)
                        nc.vector.tensor_mul(out=attnT[r0:r0 + 64, hp, :],
                                             in0=attv[0:64, :], in1=rbs)

                es_att.close()  # kT/v/qT no longer needed past attention

                # ------------ Phase D: Wo, residual, LN2, transpose --------
                if True:
                    res_d = es_d.enter_context(
                        tc.tile_pool(name="res_d", bufs=1, side="right"))
                    x1_sb = res_d.tile([128, QB, C], F32)
                    h2T = res_d.tile([128, 4, QT], BF16)
                    with tc.tile_pool(name="d_w", bufs=1) as dw, \
                         tc.tile_pool(name="d_work", bufs=3) as dwork, \
                         tc.tile_pool(name="d_psum", bufs=2, space="PSUM") as dpsum:
                        wo_sb = dw.tile([128, 4, C], BF16)
                        nc.sync.dma_start(out=wo_sb,
                                          in_=wo_d.rearrange("(g p) o -> p g o", p=128))
                        for qb in range(QB):
                            ps = dpsum.tile([128, C], F32, tag="y")
                            for cc in range(4):
                                nc.tensor.matmul(
                                    ps, attnT[:, cc, qb * 128:(qb + 1) * 128],
                                    wo_sb[:, cc, :], start=(cc == 0), stop=(cc == 3))
                            nc.vector.tensor_add(out=x1_sb[:, qb, :], in0=ps,
                                                 in1=xq_sb[:, qb, :])
                            nc.vector.tensor_add(out=x1_sb[:, qb, :],
                                                 in0=x1_sb[:, qb, :], in1=boB)
                            h2_bf = dwork.tile([128, C], BF16, tag="h2")
                            layernorm_block(dwork, x1_sb[:, qb, :], h2_bf)
                            for cc in range(4):
                                tp = dpsum.tile([128, 128], BF16, tag="tp2")
                                nc.tensor.transpose(
                                    tp, h2_bf[:, cc * 128:(cc + 1) * 128], ident)
                                nc.vector.tensor_copy(
                                    out=h2T[:, cc, qb * 128:(qb + 1) * 128], in_=tp)

                    es_attnT.close()  # attnT consumed by the Wo matmuls

                    # ------------ Phase E: feed-forward --------------------
                    with tc.tile_pool(name="e_w", bufs=1) as ew, \
                         tc.tile_pool(name="e_z", bufs=1) as ez, \
                         tc.tile_pool(name="e_work", bufs=3) as ework, \
                         tc.tile_pool(name="e_psum", bufs=4, space="PSUM") as epsum:
                        w1_sb = ew.tile([128, 4, FF], BF16)
                        nc.sync.dma_start(out=w1_sb,
                                          in_=w1_d.rearrange("(g p) o -> p g o", p=128))
                        w2_sb = ew.tile([128, 16, C], BF16)
                        nc.sync.dma_start(out=w2_sb,
                                          in_=w2_d.rearrange("(g p) o -> p g o", p=128))
                        zT = ez.tile([128, 16, QT], BF16)
                        for fb in range(16):
                            for ts in range(QT // 512):
                                ps = epsum.tile([128, 512], F32, tag="z")
                                for cc in range(4):
                                    nc.tensor.matmul(
                                        ps, w1_sb[:, cc, fb * 128:(fb + 1) * 128],
                                        h2T[:, cc, ts * 512:(ts + 1) * 512],
                                        start=(cc == 0), stop=(cc == 3))
                                nc.scalar.activation(
                                    out=zT[:, fb, ts * 512:(ts + 1) * 512], in_=ps,
                                    func=Relu, bias=b1_sb[:, fb:fb + 1], scale=1.0)
                        for qb in range(QB):
                            ps = epsum.tile([128, C], F32, tag="f")
                            for fc in range(16):
                                nc.tensor.matmul(
                                    ps, zT[:, fc, qb * 128:(qb + 1) * 128],
                                    w2_sb[:, fc, :], start=(fc == 0), stop=(fc == 15))
                            ot = ework.tile([128, C], F32, tag="ot")
                            nc.vector.tensor_add(out=ot, in0=ps,
                                                 in1=x1_sb[:, qb, :])
                            nc.vector.tensor_add(out=ot, in0=ot, in1=b2B)
                            nc.sync.dma_start(
                                out=out_d[qb * 128:(qb + 1) * 128, :], in_=ot)
                    es_d.close()

    nc.compile()
    return nc


_NC_CACHE = None


def _get_nc():
    global _NC_CACHE
    if _NC_CACHE is None:
        _NC_CACHE = build_nc()
    return _NC_CACHE


def _host_prep(inputs):
    x = np.asarray(inputs["x"], np.float32)
    g1 = np.asarray(inputs["g1"], np.float32)
    be1 = np.asarray(inputs["be1"], np.float32)
    g2 = np.asarray(inputs["g2"], np.float32)
    be2 = np.asarray(inputs["be2"], np.float32)
    Wq = np.asarray(inputs["Wq"], np.float32)
    Wk = np.asarray(inputs["Wk"], np.float32)
    Wv = np.asarray(inputs["Wv"], np.float32)
    Wo = np.asarray(inputs["Wo"], np.float32)
    W1 = np.asarray(inputs["W1"], np.float32)
    W2 = np.asarray(inputs["W2"], np.float32)

    s = float(C) ** -0.5
    wq_eff = ((g1[:, None] * Wq) * s).astype(BF16NP)
    bq_eff = ((be1 @ Wq) * s).astype(np.float32)
    wk_eff = (g1[:, None] * Wk).astype(BF16NP)
    bk_eff = (be1 @ Wk).astype(np.float32)
    wv_eff = (g1[:, None] * Wv).astype(BF16NP)
    bv_eff = (be1 @ Wv).astype(np.float32)
    w1_eff = (g2[:, None] * W1).astype(BF16NP)
    b1_eff = (np.asarray(inputs["b1"], np.float32) + be2 @ W1).astype(np.float32)

    common = {
        "wq": wq_eff, "wk": wk_eff, "wv": wv_eff,
        "wo": Wo.astype(BF16NP), "w1": w1_eff, "w2": W2.astype(BF16NP),
        "bq": bq_eff, "bk": bk_eff, "bv": bv_eff,
        "bo": np.asarray(inputs["bo"], np.float32),
        "b1": b1_eff, "b2": np.asarray(inputs["b2"], np.float32),
    }

    iii = np.arange(128)[:, None]
    jjj = np.arange(128)[None, :]
    tri = (iii <= jjj)
    in_maps = []
    qblocks_per_core = []
    for core in range(NCORES):
        b, c = divmod(core, 4)
        blocks = []
        for g in range(4):
            blocks += [8 * g + c, 8 * g + 7 - c]
        qblocks_per_core.append(blocks)
        rows = np.concatenate([np.arange(gb * 128, (gb + 1) * 128) for gb in blocks])
        masks = np.zeros((8, 128, 256), np.float32)
        for j in range(8):
            for m, cm in enumerate((c, 7 - c)):
                blk = masks[j][:, m * 128:(m + 1) * 128]
                if j < cm:
                    blk[:] = 1.0
                elif j == cm:
                    blk[:] = tri
        in_maps.append({
            "x_full": np.ascontiguousarray(x[b]),
            "xq": np.ascontiguousarray(x[b][rows]),
            "masks": masks.astype(BF16NP),
            **common,
        })
    return in_maps, qblocks_per_core


def kernel(**inputs):
    nc = _get_nc()
    in_maps, qblocks_per_core = _host_prep(inputs)
    r = run_bass_kernel_spmd(nc, in_maps, list(range(NCORES)))
    out = np.empty((B, T, C), np.float32)
    for core in range(NCORES):
        b = core // 4
        res = r.results[core]["out"]
        for vb, gb in enumerate(qblocks_per_core[core]):
            out[b, gb * 128:(gb + 1) * 128, :] = res[vb * 128:(vb + 1) * 128, :]
    return out
